# revision 1
# baseline (speedup 1.0000x reference)
"""Trainium2 Bass kernel for nn_BartDoubleTinyAttention.

Module: LayerNorm -> 1024->64 down-proj -> cross-attention (encoder KV)
        -> self-attention -> 64->1024 up-proj -> x + 0.001*h

Sharding: 8 cores = (batch b in 0..3) x (sequence half h in 0..1); each core
owns 1024 query tokens. Cross-attention is computed per-core for its own
tokens; the normalized cross-attention outputs o1 ([64, 1024] f32 per core)
are summed across the two cores of a batch pair with a 2-rank AllReduce and
each core recovers the partner half by subtracting its own. Self-attention
keys/values use the per-core KV order [own-half || other-half] (softmax is
permutation invariant over KV), which keeps the program SPMD-identical and
lets the own-half of self-attention overlap the collective.

Layout strategy (avoids all large on-chip transposes):
 - Host feeds x twice: natural fp32 (variance + residual) and transposed
   bf16 (for the 1024->64 projection, which needs features on partitions).
 - Host folds LN gain, 1/sqrt(64), wo1/wo2 and all biases into composed
   weights; the LN mean/variance correction rides as two extra contraction
   rows in the cross-attn score matmul (K=66). The token mean itself comes
   free as a ones-column of the down-projection matmul.
 - Attention tensors live "head-dim/kv-token on partitions, query tokens on
   free dim". Softmax denominators come out of the PV matmul as an extra
   ones-row of the KV matrix; 1/r is computed as exp(-log r) on the scalar
   engine (single-partition DVE reciprocal is ~6.4 ns/element) and applied
   through a K=1 ones-matmul broadcast.
"""

import math
from contextlib import ExitStack

import numpy as np
import ml_dtypes

B = 4
T_FULL = 2048
S_FULL = 2048
D_IN = 1024
DA = 64
SCALE = DA ** -0.5
EPS = 1e-5
RES_SCALE = 0.001
N_CORES = 8
P = 128

BF16 = ml_dtypes.bfloat16

_CACHE = {}


def _slices(total, step=512):
    out = []
    o = 0
    while o < total:
        sz = min(step, total - o)
        out.append((o, sz))
        o += sz
    return out


def build_program(t_own, s_full, d_in, groups):
    """Emit the SPMD bass program (identical on all cores)."""
    import concourse.bass as bass
    import concourse.tile as tile
    from concourse import bacc, mybir

    f32 = mybir.dt.float32
    bf16 = mybir.dt.bfloat16
    AF = mybir.ActivationFunctionType
    ALU = mybir.AluOpType

    FC = d_in // P            # feature chunks for the down-projection
    SC = s_full // P          # encoder kv chunks (cross attention)
    TC = t_own // P           # own-token chunks
    OC = t_own // P           # kv chunks per half (self attention)

    nc = bacc.Bacc("TRN2", target_bir_lowering=False)

    dp = nc.declare_dram_parameter
    x_own = dp("x_own", [t_own, d_in], f32, isOutput=False)
    xT_own = dp("xT_own", [d_in, t_own], bf16, isOutput=False)
    encT = dp("encT", [DA, s_full], bf16, isOutput=False)
    enc_aug = dp("enc_aug", [s_full, DA + 1], bf16, isOutput=False)
    q1_wT_aug = dp("q1_wT_aug", [d_in, DA + 1], bf16, isOutput=False)
    k1_wT_aug = dp("k1_wT_aug", [DA, DA + 2], bf16, isOutput=False)
    v1_wT = dp("v1_wT", [DA, DA], bf16, isOutput=False)
    q2_wT = dp("q2_wT", [DA, DA], bf16, isOutput=False)
    k2_wT_aug = dp("k2_wT_aug", [DA, DA + 1], bf16, isOutput=False)
    v2_wT_aug = dp("v2_wT_aug", [DA, DA + 1], bf16, isOutput=False)
    out_wT_aug = dp("out_wT_aug", [DA + 1, d_in], bf16, isOutput=False)
    k1aug_bias = dp("k1aug_bias", [DA + 2, 1], f32, isOutput=False)
    k2aug_bias = dp("k2aug_bias", [DA + 1, 1], f32, isOutput=False)
    v2_b_row = dp("v2_b_row", [1, DA + 1], f32, isOutput=False)
    ident = dp("ident", [P, P], f32, isOutput=False)
    out = dp("out", [t_own, d_in], f32, isOutput=True)

    with tile.TileContext(nc) as tc:
        with ExitStack() as ctx:
            sing = ctx.enter_context(tc.tile_pool(name="sing", bufs=1))
            bigx = ctx.enter_context(tc.tile_pool(name="bigx", bufs=1))
            work = ctx.enter_context(tc.tile_pool(name="work", bufs=3))
            outp = ctx.enter_context(tc.tile_pool(name="outp", bufs=3))
            once = ctx.enter_context(tc.tile_pool(name="once", bufs=1))
            ps_small = ctx.enter_context(
                tc.tile_pool(name="ps_small", bufs=2, space="PSUM"))
            ps_acc = ctx.enter_context(
                tc.tile_pool(name="ps_acc", bufs=1, space="PSUM"))
            ps_big = ctx.enter_context(
                tc.tile_pool(name="ps_big", bufs=2, space="PSUM"))
            dram = ctx.enter_context(
                tc.tile_pool(name="dram", bufs=1, space="DRAM"))

            # ---------------- weights / small constants first ------------
            sb_q1w = sing.tile([P, FC, DA + 1], bf16)
            nc.sync.dma_start(sb_q1w[:],
                              q1_wT_aug.rearrange("(c p) d -> p c d", p=P))
            sb_k1w = sing.tile([DA, DA + 2], bf16)
            nc.sync.dma_start(sb_k1w[:], k1_wT_aug[:])
            sb_v1w = sing.tile([DA, DA], bf16)
            nc.sync.dma_start(sb_v1w[:], v1_wT[:])
            sb_q2w = sing.tile([DA, DA], bf16)
            nc.sync.dma_start(sb_q2w[:], q2_wT[:])
            sb_k2w = sing.tile([DA, DA + 1], bf16)
            nc.sync.dma_start(sb_k2w[:], k2_wT_aug[:])
            sb_v2w = sing.tile([DA, DA + 1], bf16)
            nc.sync.dma_start(sb_v2w[:], v2_wT_aug[:])
            sb_outw = sing.tile([DA + 1, d_in], bf16)
            nc.sync.dma_start(sb_outw[:], out_wT_aug[:])
            sb_k1b = sing.tile([DA + 2, 1], f32)
            nc.sync.dma_start(sb_k1b[:], k1aug_bias[:])
            sb_k2b = sing.tile([DA + 1, 1], f32)
            nc.sync.dma_start(sb_k2b[:], k2aug_bias[:])
            sb_v2b = sing.tile([P, DA + 1], f32)
            v2b_ap = v2_b_row[:]
            v2b_bcast = bass.AP(
                tensor=v2b_ap.tensor, offset=v2b_ap.offset,
                ap=[[0, P], [1, DA + 1]])
            nc.sync.dma_start(sb_v2b[:], v2b_bcast)
            sb_ident_dma = sing.tile([P, P], f32)
            nc.sync.dma_start(sb_ident_dma[:], ident[:])
            sb_ident = sing.tile([P, P], f32)
            nc.vector.tensor_copy(out=sb_ident[:], in_=sb_ident_dma[:])
            sb_eps = sing.tile([1, 1], f32)
            nc.vector.memset(sb_eps[:], EPS)
            sb_ones64 = sing.tile([1, DA], bf16)
            nc.vector.memset(sb_ones64[:], 1.0)

            def bcast64(row_f32, tag):
                """Broadcast a [1, N] f32 sbuf row to a [64, N] f32 sbuf tile
                via a K=1 matmul with a ones stationary (PSUM bounce)."""
                n = row_f32.shape[-1]
                row_bf = once.tile([1, n], bf16, tag="row_bf")
                nc.vector.tensor_copy(out=row_bf[:], in_=row_f32)
                pb = ps_big.tile([DA, n], f32, tag="ps_big")
                for (ns, nsz) in _slices(n):
                    nc.tensor.matmul(pb[:, ns:ns + nsz], sb_ones64[:],
                                     row_bf[:, ns:ns + nsz],
                                     start=True, stop=True)
                sb = once.tile([DA, n], f32, tag="bc_sb")
                nc.vector.tensor_copy(out=sb[:], in_=pb[:])
                return sb

            def rcp_row(row_ps, tag):
                """1/row via exp(-log(row)) on the scalar engine."""
                lg = once.tile([1, row_ps.shape[-1]], f32, tag="row_lg")
                nc.scalar.activation(out=lg[:], in_=row_ps, func=AF.Ln)
                rc = sing.tile([1, row_ps.shape[-1]], f32, tag=tag + "_rc")
                nc.scalar.activation(out=rc[:], in_=lg[:], func=AF.Exp,
                                     scale=-1.0)
                return rc

            # ---------------- big input loads (xT before x) ---------------
            sb_xT = bigx.tile([P, FC, t_own], bf16)
            nc.scalar.dma_start(sb_xT[:], xT_own.rearrange("(c p) t -> p c t", p=P))
            sb_encT = bigx.tile([DA, s_full], bf16)
            nc.sync.dma_start(sb_encT[:], encT[:])
            sb_enc = bigx.tile([P, SC, DA + 1], bf16)
            nc.sync.dma_start(sb_enc[:],
                              enc_aug.rearrange("(c p) d -> p c d", p=P))
            xr = x_own.rearrange("(c p) d -> p c d", p=P)
            x_tiles = []
            ssq_cols = []
            for i in range(TC):
                xt = bigx.tile([P, d_in], f32, tag=f"x{i}")
                nc.scalar.dma_start(xt[:], xr[:, i, :])
                x_tiles.append(xt)
                sq = work.tile([P, d_in], f32, tag="sq")
                sc_ = once.tile([P, 1], f32, tag=f"ssq{i}")
                nc.vector.tensor_mul(sq[:], xt[:], xt[:])
                nc.vector.reduce_sum(out=sc_[:], in_=sq[:],
                                     axis=mybir.AxisListType.X)
                ssq_cols.append(sc_)

            # ---------------- q1 projection (mean rides as row 64) --------
            ps_q1 = ps_acc.tile([DA + 1, t_own], f32, tag="ps_acc")
            for (ns, nsz) in _slices(t_own):
                for c in range(FC):
                    nc.tensor.matmul(ps_q1[:, ns:ns + nsz], sb_q1w[:, c, :],
                                     sb_xT[:, c, ns:ns + nsz],
                                     start=(c == 0), stop=(c == FC - 1))

            # ---------------- LayerNorm stats (row-space) -----------------
            # ssq_row[t] = sum_f x[t,f]^2 ; mu_row = ps_q1[64]/D
            ssq_row = sing.tile([1, t_own], f32)
            for i in range(TC):
                pta = ps_small.tile([1, P], f32, tag="ps_small")
                nc.tensor.transpose(pta[:], ssq_cols[i][:], sb_ident[:])
                nc.vector.tensor_copy(out=ssq_row[:, i * P:(i + 1) * P],
                                      in_=pta[:])
            mu_row = sing.tile([1, t_own], f32)
            nc.vector.tensor_scalar_mul(mu_row[:], ps_q1[DA:DA + 1, :],
                                        1.0 / d_in)
            mu2_row = once.tile([1, t_own], f32, tag="row_a")
            nc.vector.tensor_mul(mu2_row[:], mu_row[:], mu_row[:])
            var_row = once.tile([1, t_own], f32, tag="row_b")
            nc.vector.tensor_scalar_mul(var_row[:], ssq_row[:], 1.0 / d_in)
            nc.vector.tensor_tensor(out=var_row[:], in0=var_row[:],
                                    in1=mu2_row[:], op=ALU.subtract)
            # rsig = exp(-0.5 * log(var + eps))
            lgv = once.tile([1, t_own], f32, tag="row_a")
            nc.scalar.activation(out=lgv[:], in_=var_row[:], func=AF.Ln,
                                 bias=sb_eps[:])
            rsig_row = sing.tile([1, t_own], f32)
            nc.scalar.activation(out=rsig_row[:], in_=lgv[:], func=AF.Exp,
                                 scale=-0.5)
            m2_row = sing.tile([1, t_own], f32)
            nc.vector.tensor_mul(m2_row[:], mu_row[:], rsig_row[:])

            rsig_b = bcast64(rsig_row[:], "rsig")
            q1aug = sing.tile([DA + 2, t_own], bf16)
            nc.vector.tensor_mul(q1aug[0:DA, :], ps_q1[0:DA, :], rsig_b[:])
            nc.vector.memset(q1aug[DA:DA + 2, :], 1.0)
            nc.vector.tensor_copy(out=q1aug[DA:DA + 1, :], in_=m2_row[:])

            # ---------------- K1 (cross attention keys, augmented) --------
            k1aug = sing.tile([DA + 2, s_full], bf16)
            for (ns, nsz) in _slices(s_full):
                pk = ps_small.tile([DA + 2, nsz], f32, tag="ps_small")
                nc.tensor.matmul(pk[:], sb_k1w[:], sb_encT[:, ns:ns + nsz],
                                 start=True, stop=True)
                nc.vector.tensor_scalar_add(k1aug[:, ns:ns + nsz], pk[:],
                                            sb_k1b[:])

            # ---------------- cross attention ----------------
            ps_mix = ps_acc.tile([DA + 1, t_own], f32, tag="ps_acc")
            for sc in range(SC):
                ps_s = ps_big.tile([P, t_own], f32, tag="ps_big")
                for (ns, nsz) in _slices(t_own):
                    nc.tensor.matmul(ps_s[:, ns:ns + nsz],
                                     k1aug[:, sc * P:(sc + 1) * P],
                                     q1aug[:, ns:ns + nsz],
                                     start=True, stop=True)
                a1 = work.tile([P, t_own], bf16, tag="a_t")
                nc.scalar.activation(out=a1[:], in_=ps_s[:], func=AF.Exp)
                for (ns, nsz) in _slices(t_own):
                    nc.tensor.matmul(ps_mix[:, ns:ns + nsz], sb_enc[:, sc, :],
                                     a1[:, ns:ns + nsz],
                                     start=(sc == 0), stop=(sc == SC - 1))

            # w1maug rows 0-63: enc-mixed attention numerator; row 64: r1.
            w1maug = sing.tile([DA + 1, t_own], bf16)
            nc.vector.tensor_copy(out=w1maug[:], in_=ps_mix[:])

            # ---------------- pair exchange of [w1m || r1] (AllReduce) ----
            # Issued as early as possible; each core reconstructs the
            # partner's half by subtracting its own contribution.
            cc_in = dram.tile([DA + 1, t_own], bf16)
            cc_out = dram.tile([DA + 1, t_own], bf16)
            nc.sync.dma_start(cc_in[:], w1maug[:])
            nc.gpsimd.collective_compute(
                "AllReduce", mybir.AluOpType.add, replica_groups=groups,
                ins=[cc_in.opt()], outs=[cc_out.opt()])

            def finish_o1(w1m_aug_bf, tag):
                """v1 projection + softmax normalization from a [w1m||r1]."""
                rc = rcp_row(w1m_aug_bf[DA:DA + 1, :], tag)
                rc_b = bcast64(rc[:], tag)
                o1r = sing.tile([DA, t_own], bf16, tag=tag + "_o1r")
                for (ns, nsz) in _slices(t_own):
                    ps_o1 = ps_small.tile([DA, nsz], f32, tag="ps_small")
                    nc.tensor.matmul(ps_o1[:], sb_v1w[:],
                                     w1m_aug_bf[0:DA, ns:ns + nsz],
                                     start=True, stop=True)
                    nc.vector.tensor_mul(o1r[:, ns:ns + nsz], ps_o1[:],
                                         rc_b[:, ns:ns + nsz])
                return o1r

            o1r_bf = finish_o1(w1maug, "rcp1")

            # -------- self attention prep + own half (overlaps collective)
            k2aug = sing.tile([DA + 1, 2 * t_own], bf16)
            q2aug = sing.tile([DA + 1, t_own], bf16)
            v2aug = sing.tile([P, 2 * OC, DA + 1], bf16)

            def k2_half(src_bf, off):
                for (ns, nsz) in _slices(t_own):
                    pk2 = ps_small.tile([DA + 1, nsz], f32, tag="ps_small")
                    nc.tensor.matmul(pk2[:], sb_k2w[:], src_bf[:, ns:ns + nsz],
                                     start=True, stop=True)
                    nc.vector.tensor_scalar_add(
                        k2aug[:, off + ns:off + ns + nsz], pk2[:], sb_k2b[:])

            def v2_chunks(src_bf, sc0):
                for c in range(OC):
                    pv2 = ps_small.tile([P, DA + 1], f32, tag="ps_small")
                    nc.tensor.matmul(pv2[:], src_bf[:, c * P:(c + 1) * P],
                                     sb_v2w[:], start=True, stop=True)
                    nc.vector.tensor_add(v2aug[:, sc0 + c, :], pv2[:], sb_v2b[:])

            for (ns, nsz) in _slices(t_own):
                pq2 = ps_small.tile([DA, nsz], f32, tag="ps_small")
                nc.tensor.matmul(pq2[:], sb_q2w[:], o1r_bf[:, ns:ns + nsz],
                                 start=True, stop=True)
                nc.vector.tensor_copy(out=q2aug[0:DA, ns:ns + nsz], in_=pq2[:])
            nc.vector.memset(q2aug[DA:DA + 1, :], 1.0)
            k2_half(o1r_bf[:], 0)
            v2_chunks(o1r_bf[:], 0)

            ps_o2 = ps_acc.tile([DA + 1, t_own], f32, tag="ps_acc")

            def self_attn_chunks(sc_list, start_sc, stop_sc):
                for sc in sc_list:
                    ps_s2 = ps_big.tile([P, t_own], f32, tag="ps_big")
                    for (ns, nsz) in _slices(t_own):
                        nc.tensor.matmul(ps_s2[:, ns:ns + nsz],
                                         k2aug[:, sc * P:(sc + 1) * P],
                                         q2aug[:, ns:ns + nsz],
                                         start=True, stop=True)
                    a2 = work.tile([P, t_own], bf16, tag="a_t")
                    nc.scalar.activation(out=a2[:], in_=ps_s2[:], func=AF.Exp)
                    for (ns, nsz) in _slices(t_own):
                        nc.tensor.matmul(ps_o2[:, ns:ns + nsz],
                                         v2aug[:, sc, :],
                                         a2[:, ns:ns + nsz],
                                         start=(sc == start_sc),
                                         stop=(sc == stop_sc))

            self_attn_chunks(range(OC), 0, 2 * OC - 1)

            # -------- other half arrives: sum - own = other ---------------
            sum_sb = sing.tile([DA + 1, t_own], bf16)
            nc.sync.dma_start(sum_sb[:], cc_out[:])
            w1m_oth = sing.tile([DA + 1, t_own], bf16)
            nc.vector.tensor_tensor(out=w1m_oth[:], in0=sum_sb[:],
                                    in1=w1maug[:], op=ALU.subtract)
            oth_bf = finish_o1(w1m_oth, "rcp1o")
            k2_half(oth_bf[:], t_own)
            v2_chunks(oth_bf[:], OC)
            self_attn_chunks(range(OC, 2 * OC), 0, 2 * OC - 1)

            # ---------------- normalize o2, output projection -------------
            rcp2 = rcp_row(ps_o2[DA:DA + 1, :], "rcp2")
            rcp2_b = bcast64(rcp2[:], "rcp2")
            o2n = sing.tile([DA + 1, t_own], bf16)
            nc.vector.tensor_mul(o2n[0:DA, :], ps_o2[0:DA, :], rcp2_b[:])
            nc.vector.memset(o2n[DA:DA + 1, :], 1.0)

            out_r = out.rearrange("(c p) d -> p c d", p=P)
            for i in range(TC):
                po = ps_big.tile([P, d_in], f32, tag="ps_big")
                for (ns, nsz) in _slices(d_in):
                    nc.tensor.matmul(po[:, ns:ns + nsz],
                                     o2n[:, i * P:(i + 1) * P],
                                     sb_outw[:, ns:ns + nsz],
                                     start=True, stop=True)
                ot = outp.tile([P, d_in], f32, tag="ot")
                nc.vector.tensor_add(ot[:], po[:], x_tiles[i][:])
                nc.sync.dma_start(out_r[:, i, :], ot[:])

    nc.compile()
    return nc


def prep_weights(f):
    """Host-side composition of the tiny weight matrices (all fp32 numpy)."""
    g, bl = f["ln_g"], f["ln_b"]
    w1g = f["w1"] * g[None, :]
    c1 = f["w1"] @ bl + f["b1"]
    q1_w = SCALE * (f["wq1"] @ w1g)                     # [64, D]
    q1_b = SCALE * (f["wq1"] @ c1 + f["bq1"])           # [64]
    s1 = q1_w.sum(axis=1)                               # [64]

    da = DA
    d_in = f["w1"].shape[1]
    q1_wT_aug = np.ones((d_in, da + 1), np.float32)
    q1_wT_aug[:, 0:da] = q1_w.T

    k1_wT_aug = np.zeros((da, da + 2), np.float32)
    k1_wT_aug[:, 0:da] = f["wk1"].T
    k1_wT_aug[:, da] = f["wk1"].T @ (-s1)
    k1_wT_aug[:, da + 1] = f["wk1"].T @ q1_b
    k1aug_bias = np.concatenate(
        [f["bk1"], [-(f["bk1"] @ s1)], [f["bk1"] @ q1_b]]).astype(np.float32)[:, None]

    # fold wo1 and the v1/wo1 biases into the q2/k2/v2 path.
    # o1r (on-device) = softmax(scores1) @ (enc @ wv1.T)  [no bv1]
    # h_mid = (o1r + bv1) @ wo1.T + bo1
    v1b_fold = f["wo1"] @ f["bv1"] + f["bo1"]           # [64]
    q2_w = SCALE * (f["wq2"] @ f["wo1"])
    q2_b = SCALE * (f["wq2"] @ v1b_fold + f["bq2"])
    k2_w = f["wk2"] @ f["wo1"]
    k2_b = f["wk2"] @ v1b_fold + f["bk2"]
    v2_w = f["wv2"] @ f["wo1"]
    v2_b = f["wv2"] @ v1b_fold + f["bv2"]

    k2_wT_aug = np.zeros((da, da + 1), np.float32)
    k2_wT_aug[:, 0:da] = k2_w.T
    k2_wT_aug[:, da] = k2_w.T @ q2_b
    k2aug_bias = np.concatenate([k2_b, [k2_b @ q2_b]]).astype(np.float32)[:, None]

    v2_wT_aug = np.zeros((da, da + 1), np.float32)
    v2_wT_aug[:, 0:da] = v2_w.T
    v2_b_row = np.concatenate([v2_b, [1.0]]).astype(np.float32)[None, :]

    out_w = RES_SCALE * (f["w2"] @ f["wo2"])            # [D, 64]
    out_b = RES_SCALE * (f["w2"] @ f["bo2"] + f["b2"])  # [D]
    out_wT_aug = np.zeros((da + 1, d_in), np.float32)
    out_wT_aug[0:da, :] = out_w.T
    out_wT_aug[da, :] = out_b

    bf = lambda a: np.ascontiguousarray(a).astype(BF16)
    return {
        "q1_wT_aug": bf(q1_wT_aug),
        "k1_wT_aug": bf(k1_wT_aug),
        "v1_wT": bf(f["wv1"].T),
        "q2_wT": bf(q2_w.T),
        "k2_wT_aug": bf(k2_wT_aug),
        "v2_wT_aug": bf(v2_wT_aug),
        "out_wT_aug": bf(out_wT_aug),
        "k1aug_bias": k1aug_bias,
        "k2aug_bias": k2aug_bias,
        "v2_b_row": v2_b_row,
        "ident": np.eye(P, dtype=np.float32),
    }


def make_in_maps(inputs, t_own=T_FULL // 2):
    """Build the per-core input dicts from the full problem inputs."""
    f = {k: np.asarray(v, np.float32) for k, v in inputs.items()}
    w = prep_weights(f)
    x = f["hidden_states"]
    enc = f["encoder_hidden_states"]
    b_count = x.shape[0]
    in_maps = []
    for c in range(2 * b_count):
        b, h = c // 2, c % 2
        xo = np.ascontiguousarray(x[b, h * t_own:(h + 1) * t_own, :])
        m = dict(w)
        m["x_own"] = xo
        m["xT_own"] = np.ascontiguousarray(xo.T).astype(BF16)
        m["encT"] = np.ascontiguousarray(enc[b].T).astype(BF16)
        ea = np.ones((enc.shape[1], DA + 1), np.float32)
        ea[:, 0:DA] = enc[b]
        m["enc_aug"] = ea.astype(BF16)
        in_maps.append(m)
    return in_maps


LAST_RESULT = None


def kernel(**inputs):
    global LAST_RESULT
    from concourse.bass_utils import run_bass_kernel_spmd

    t_own = T_FULL // 2
    groups = [[0, 1], [2, 3], [4, 5], [6, 7]]
    key = (t_own, S_FULL, D_IN)
    if key not in _CACHE:
        _CACHE[key] = build_program(t_own, S_FULL, D_IN, groups)
    nc = _CACHE[key]

    in_maps = make_in_maps(inputs, t_own)
    res = run_bass_kernel_spmd(nc, in_maps, core_ids=list(range(N_CORES)))
    LAST_RESULT = res

    out = np.empty((B, T_FULL, D_IN), dtype=np.float32)
    for c in range(N_CORES):
        b, h = c // 2, c % 2
        out[b, h * t_own:(h + 1) * t_own, :] = res.results[c]["out"]
    return out



# revision 17
# speedup vs baseline: 1.0680x; 1.0680x over previous
"""Trainium2 Bass kernel for nn_BartDoubleTinyAttention.

Module: LayerNorm -> 1024->64 down-proj -> cross-attention (encoder KV)
        -> self-attention -> 64->1024 up-proj -> x + 0.001*h

Sharding: 8 cores = (batch b in 0..3) x (sequence half h in 0..1); each core
owns 1024 query tokens. The cross-attention numerator/denominator mix
([64 mix + r1] = [65, 1024] bf16) is summed across the two cores of a batch
pair with a 2-rank AllReduce; each core recovers the partner half by
subtracting its own. Self-attention uses per-core KV order [own || other]
(softmax is KV-permutation invariant).

Key structure (vs a straightforward port):
 - wo1/wv1 and every bias are folded on the host into composed matrices;
   the self-attention q2/k2/v2 projections read the *unnormalized* cross
   mix w1m directly, with the softmax denominator r1 riding as an extra
   contraction row so all biases stay exact.
 - The 1/r1 normalization of self-attention KV happens inside the exp via
   the Activation engine's per-partition scale/bias operands (and fused
   scalar ops on DVE tiles); only the query side needs one explicit
   broadcast-multiply.
 - exp is split between the Activation engine (table exp) and the Vector
   engine (Schraudolph: one fused tensor_scalar writing int16 bits that are
   re-read as bf16).
 - The final softmax normalization (1/r2) and the residual add are fused
   into one DVE scalar_tensor_tensor per output tile, using r2 transposed
   into per-partition columns; the up-projection bias rides the r2 row of
   the stationary so it comes out exact.
 - LayerNorm stats via DVE bn_stats/bn_aggr on the natural-layout bf16 x;
   the Act engine does a single Rsqrt; k1 biases fold via a ones-row in the
   host-fed transposed encoder.
 - Attention loops are software-pipelined (scores for chunk c+1 issue
   before the PV matmul of chunk c) so the PE never stalls on an exp.
"""

import math
from contextlib import ExitStack

import numpy as np
import ml_dtypes

B = 4
T_FULL = 2048
S_FULL = 2048
D_IN = 1024
DA = 64
SCALE = DA ** -0.5
EPS = 1e-5
RES_SCALE = 0.001
N_CORES = 8
P = 128

BF16 = ml_dtypes.bfloat16

# Schraudolph exp constants (bf16 bit space): i16 = round(s*EXPA + EXPB),
# bits reinterpreted as bf16 give exp(s) to ~3%.
EXPA = 184.6650558  # log2(e) * 2^7
EXPB = 16252.0      # 127 * 2^7 - 4 (balanced error)
# ln approximation (fp32 bit space): ln(x) ~ (i32(x) - B)*LNK + 0.03
LNK = 8.2629582e-8  # ln2 / 2^23
LNC = 88.02969193 - 0.03  # B*LNK - correction

_CACHE = {}


def _slices(total, step=512):
    out = []
    o = 0
    while o < total:
        sz = min(step, total - o)
        out.append((o, sz))
        o += sz
    return out


def build_program(t_own, s_full, d_in, groups, n_act_exp1=9, n_act_exp2=4):
    """Emit the SPMD bass program (identical on all cores)."""
    import concourse.bass as bass
    import concourse.tile as tile
    from concourse import bacc, mybir

    f32 = mybir.dt.float32
    bf16 = mybir.dt.bfloat16
    i16 = mybir.dt.int16
    i32 = mybir.dt.int32
    AF = mybir.ActivationFunctionType
    ALU = mybir.AluOpType

    FC = d_in // P            # feature chunks for the down-projection
    SC = s_full // P          # encoder kv chunks (cross attention)
    TC = t_own // P           # own-token chunks
    OC = t_own // P           # kv chunks per half (self attention)

    nc = bacc.Bacc("TRN2", target_bir_lowering=False)

    dp = nc.declare_dram_parameter
    x_nat = dp("x_nat", [t_own, d_in], bf16, isOutput=False)
    xT = dp("xT", [d_in, t_own], bf16, isOutput=False)
    encTa = dp("encTa", [DA + 1, s_full], bf16, isOutput=False)
    enca = dp("enca", [s_full, DA + 1], bf16, isOutput=False)
    q1s = dp("q1s", [d_in, DA], bf16, isOutput=False)
    k1s = dp("k1s", [DA + 1, DA + 2], bf16, isOutput=False)
    q2s = dp("q2s", [DA, DA], bf16, isOutput=False)
    k2s = dp("k2s", [DA + 1, DA + 1], bf16, isOutput=False)
    v2s = dp("v2s", [DA + 1, DA + 1], bf16, isOutput=False)
    outw = dp("outw", [DA + 1, d_in], bf16, isOutput=False)
    ident = dp("ident", [P, P], bf16, isOutput=False)
    out = dp("out", [t_own, d_in], f32, isOutput=True)

    with tile.TileContext(nc) as tc:
        with ExitStack() as ctx:
            sing = ctx.enter_context(tc.tile_pool(name="sing", bufs=1))
            bigx = ctx.enter_context(tc.tile_pool(name="bigx", bufs=1))
            work = ctx.enter_context(tc.tile_pool(name="work", bufs=3))
            outp = ctx.enter_context(tc.tile_pool(name="outp", bufs=3))
            once = ctx.enter_context(tc.tile_pool(name="once", bufs=2))
            ps_small = ctx.enter_context(
                tc.tile_pool(name="ps_small", bufs=2, space="PSUM"))
            ps_acc = ctx.enter_context(
                tc.tile_pool(name="ps_acc", bufs=1, space="PSUM"))
            ps_big = ctx.enter_context(
                tc.tile_pool(name="ps_big", bufs=2, space="PSUM"))
            dram = ctx.enter_context(
                tc.tile_pool(name="dram", bufs=1, space="DRAM"))

            # ---------------- weights / constants (sync queue) -----------
            sb_q1s = sing.tile([P, FC, DA], bf16)
            nc.sync.dma_start(sb_q1s[:], q1s.rearrange("(c p) m -> p c m", p=P))
            sb_k1s = sing.tile([DA + 1, DA + 2], bf16)
            nc.sync.dma_start(sb_k1s[:], k1s[:])
            sb_q2s = sing.tile([DA, DA], bf16)
            nc.sync.dma_start(sb_q2s[:], q2s[:])
            sb_k2s = sing.tile([DA + 1, DA + 1], bf16)
            nc.sync.dma_start(sb_k2s[:], k2s[:])
            sb_v2s = sing.tile([DA + 1, DA + 1], bf16)
            nc.sync.dma_start(sb_v2s[:], v2s[:])
            sb_outw = sing.tile([DA + 1, d_in], bf16)
            nc.sync.dma_start(sb_outw[:], outw[:])
            sb_ident = sing.tile([P, P], bf16)
            nc.sync.dma_start(sb_ident[:], ident[:])
            sb_identf = sing.tile([P, P], f32)
            nc.vector.tensor_copy(out=sb_identf[:], in_=sb_ident[:])
            sb_eps = sing.tile([1, 1], f32)
            nc.vector.memset(sb_eps[:], EPS)
            sb_ones64 = sing.tile([1, DA], bf16)
            nc.vector.memset(sb_ones64[:], 1.0)
            sb_onecol = sing.tile([DA + 1, 1], bf16)
            nc.vector.memset(sb_onecol[:], 1.0)
            sb_one1 = sb_onecol[DA:DA + 1, :]

            # ---------------- big input loads ----------------------------
            sb_xT = bigx.tile([P, FC, t_own], bf16)
            nc.scalar.dma_start(sb_xT[:], xT.rearrange("(c p) t -> p c t", p=P))
            sb_encTa = bigx.tile([DA + 1, s_full], bf16)
            nc.sync.dma_start(sb_encTa[:], encTa[:])
            sb_enca = bigx.tile([P, SC, DA + 1], bf16)
            nc.sync.dma_start(sb_enca[:],
                              enca.rearrange("(c p) d -> p c d", p=P))
            xr = x_nat.rearrange("(c p) d -> p c d", p=P)
            x_tiles = []
            for i in range(TC):
                xt = bigx.tile([P, d_in], bf16, tag=f"x{i}")
                nc.gpsimd.dma_start(xt[:], xr[:, i, :])
                x_tiles.append(xt)

            # ---------------- LayerNorm stats (bn_stats per tile) --------
            statcols = sing.tile([P, 2 * TC], f32)
            for i in range(TC):
                bno = once.tile([P, 12], f32, tag="bno")
                nc.vector.bn_stats(bno[:, 0:6], x_tiles[i][:, 0:512])
                nc.vector.bn_stats(bno[:, 6:12], x_tiles[i][:, 512:1024])
                nc.vector.bn_aggr(statcols[:, 2 * i:2 * i + 2], bno[:])
            mu_row_t = sing.tile([1, t_own], f32)
            var_row_t = sing.tile([1, t_own], f32)
            for i in range(TC):
                for j, dst in ((0, mu_row_t), (1, var_row_t)):
                    pst = ps_small.tile([1, P], f32, tag="ps_small")
                    nc.tensor.transpose(
                        pst[:], statcols[:, 2 * i + j:2 * i + j + 1],
                        sb_identf[:])
                    nc.vector.tensor_copy(out=dst[:, i * P:(i + 1) * P],
                                          in_=pst[:])
            mu_row = mu_row_t[:]
            var_row = var_row_t[:]
            lgv_row = once.tile([1, t_own], f32, tag="lgv")
            nc.scalar.activation(out=lgv_row[:], in_=var_row, func=AF.Ln,
                                 bias=sb_eps[:])
            rsig_row = sing.tile([1, t_own], bf16)
            nc.scalar.activation(out=rsig_row[:], in_=lgv_row[:], func=AF.Exp,
                                 scale=-0.5)
            m2_row = sing.tile([1, t_own], bf16)
            nc.vector.tensor_mul(m2_row[:], mu_row, rsig_row[:])

            # ---------------- down-projection (raw q1) -------------------
            ps_q1 = ps_acc.tile([DA, t_own], f32, tag="ps_acc")
            for (ns, nsz) in _slices(t_own):
                for c in range(FC):
                    nc.tensor.matmul(ps_q1[:, ns:ns + nsz], sb_q1s[:, c, :],
                                     sb_xT[:, c, ns:ns + nsz],
                                     start=(c == 0), stop=(c == FC - 1))

            # rsig broadcast to 64 partitions via ones-matmul + Act copy
            ps_rb = ps_big.tile([P, t_own], f32, tag="ps_big")
            for (ns, nsz) in _slices(t_own):
                nc.tensor.matmul(ps_rb[0:DA, ns:ns + nsz], sb_ones64[:],
                                 rsig_row[:, ns:ns + nsz],
                                 start=True, stop=True)
            rsig_b = sing.tile([DA, t_own], bf16)
            nc.scalar.activation(out=rsig_b[:], in_=ps_rb[0:DA, :],
                                 func=AF.Copy)

            # q1aug: rows 0-63 = rsig*q1raw, row 64 = mu*rsig, row 65 = 1
            q1aug = sing.tile([DA + 2, t_own], bf16)
            nc.vector.tensor_mul(q1aug[0:DA, :], ps_q1[:], rsig_b[:])
            nc.vector.memset(q1aug[DA:DA + 2, :], 1.0)
            nc.vector.tensor_copy(out=q1aug[DA:DA + 1, :], in_=m2_row[:])

            # ---------------- K1 keys (biases folded via ones-row) -------
            k1aug = sing.tile([DA + 2, s_full], bf16)
            for (ns, nsz) in _slices(s_full):
                pk = ps_small.tile([DA + 2, nsz], f32, tag="ps_small")
                nc.tensor.matmul(pk[:], sb_k1s[:], sb_encTa[:, ns:ns + nsz],
                                 start=True, stop=True)
                nc.scalar.activation(out=k1aug[:, ns:ns + nsz], in_=pk[:],
                                     func=AF.Copy)

            # ---------------- cross attention (pipelined chunks) ---------
            ps_mix = ps_acc.tile([DA + 1, t_own], f32, tag="ps_acc")
            sl = _slices(t_own)

            def scores1(c):
                ps_s = ps_big.tile([P, t_own], f32, tag="ps_big")
                for (ns, nsz) in sl:
                    nc.tensor.matmul(ps_s[:, ns:ns + nsz],
                                     k1aug[:, c * P:(c + 1) * P],
                                     q1aug[:, ns:ns + nsz],
                                     start=True, stop=True)
                return ps_s

            def exp1(ps_s, c):
                a1 = work.tile([P, t_own], bf16, tag="a_t")
                if c < n_act_exp1:
                    nc.scalar.activation(out=a1[:], in_=ps_s[:], func=AF.Exp)
                else:
                    nc.vector.tensor_scalar(
                        out=a1[:].bitcast(i16), in0=ps_s[:],
                        scalar1=EXPA, scalar2=EXPB,
                        op0=ALU.mult, op1=ALU.add)
                return a1

            def pv1(a1, c):
                for (ns, nsz) in sl:
                    nc.tensor.matmul(ps_mix[:, ns:ns + nsz], sb_enca[:, c, :],
                                     a1[:, ns:ns + nsz],
                                     start=(c == 0), stop=(c == SC - 1))

            prev = None
            for c in range(SC):
                ps_s = scores1(c)
                if prev is not None:
                    pv1(exp1(*prev), prev[1])
                prev = (ps_s, c)
            pv1(exp1(*prev), prev[1])

            # mix + r1 row -> bf16, exchange with partner core
            w1maug = sing.tile([DA + 1, t_own], bf16)
            nc.scalar.activation(out=w1maug[:], in_=ps_mix[:], func=AF.Copy)
            cc_in = dram.tile([DA + 1, t_own], bf16)
            cc_out = dram.tile([DA + 1, t_own], bf16)
            nc.sync.dma_start(cc_in[:], w1maug[:])
            nc.gpsimd.collective_compute(
                "AllReduce", mybir.AluOpType.add, replica_groups=groups,
                ins=[cc_in.opt()], outs=[cc_out.opt()])

            # ---------------- self attention: own-half prep --------------
            # q2 (query side, explicitly normalized by 1/r1[t])
            rc1_row = sing.tile([1, t_own], bf16)
            with nc.allow_low_precision(reason="1/r1 only scales softmax "
                                        "weights; bf16 is ample here"):
                nc.vector.reciprocal(rc1_row[:], w1maug[DA:DA + 1, :])
            ps_rc = ps_big.tile([P, t_own], f32, tag="ps_big")
            for (ns, nsz) in sl:
                nc.tensor.matmul(ps_rc[0:DA, ns:ns + nsz], sb_ones64[:],
                                 rc1_row[:, ns:ns + nsz],
                                 start=True, stop=True)
            rc1_b = sing.tile([DA, t_own], bf16)
            nc.scalar.activation(out=rc1_b[:], in_=ps_rc[0:DA, :],
                                 func=AF.Copy)
            ps_q2 = ps_big.tile([P, t_own], f32, tag="ps_big")
            for (ns, nsz) in sl:
                nc.tensor.matmul(ps_q2[0:DA, ns:ns + nsz], sb_q2s[:],
                                 w1maug[0:DA, ns:ns + nsz],
                                 start=True, stop=True)
            q2aug = sing.tile([DA + 1, t_own], bf16)
            nc.vector.tensor_mul(q2aug[0:DA, :], ps_q2[0:DA, :], rc1_b[:])
            nc.vector.memset(q2aug[DA:DA + 1, :], 1.0)

            k2raw = sing.tile([DA + 1, 2 * t_own], bf16)
            v2raw = sing.tile([P, 2 * OC, DA + 1], bf16)
            r1cols = sing.tile([P, 2 * OC], f32)
            sc_a = sing.tile([P, 2 * OC], f32)   # act exp scale (1/r1)
            sc_b = sing.tile([P, 2 * OC], f32)   # act exp bias (-ln r1)
            sd_a = sing.tile([P, 2 * OC], f32)   # dve exp scale (EXPA/r1)
            sd_b = sing.tile([P, 2 * OC], f32)   # dve exp bias

            def prep_half(src, off):
                """k2raw/v2raw/r1cols + exp scale/bias columns for one half.
                src = [65, t_own] bf16 (rows 0-63 mix, row 64 r1)."""
                for (ns, nsz) in sl:
                    pk2 = ps_small.tile([DA + 1, nsz], f32, tag="ps_small")
                    nc.tensor.matmul(pk2[:], sb_k2s[:], src[:, ns:ns + nsz],
                                     start=True, stop=True)
                    nc.scalar.activation(
                        out=k2raw[:, off * t_own + ns:off * t_own + ns + nsz],
                        in_=pk2[:], func=AF.Copy)
                for c in range(OC):
                    pv2 = ps_small.tile([P, DA + 1], f32, tag="ps_small")
                    nc.tensor.matmul(pv2[:], src[:, c * P:(c + 1) * P],
                                     sb_v2s[:], start=True, stop=True)
                    nc.vector.tensor_copy(out=v2raw[:, off * OC + c, :],
                                          in_=pv2[:])
                    pr = ps_small.tile([P, 1], bf16, tag="ps_small")
                    nc.tensor.transpose(pr[:],
                                        src[DA:DA + 1, c * P:(c + 1) * P],
                                        sb_one1)
                    nc.vector.tensor_copy(
                        out=r1cols[:, off * OC + c:off * OC + c + 1],
                        in_=pr[:])
                cs = slice(off * OC, off * OC + OC)
                nc.vector.reciprocal(sc_a[:, cs], r1cols[:, cs])
                nc.vector.tensor_scalar(
                    out=sc_b[:, cs], in0=r1cols[:, cs].bitcast(i32),
                    scalar1=-LNK, scalar2=LNC, op0=ALU.mult, op1=ALU.add)
                nc.vector.tensor_scalar_mul(sd_a[:, cs], sc_a[:, cs], EXPA)
                nc.vector.tensor_scalar(
                    out=sd_b[:, cs], in0=sc_b[:, cs],
                    scalar1=EXPA, scalar2=EXPB, op0=ALU.mult, op1=ALU.add)

            prep_half(w1maug[:], 0)

            ps_o2 = ps_acc.tile([DA + 1, t_own], f32, tag="ps_acc")

            def scores2(c):
                ps_s2 = ps_big.tile([P, t_own], f32, tag="ps_big")
                for (ns, nsz) in sl:
                    nc.tensor.matmul(ps_s2[:, ns:ns + nsz],
                                     k2raw[:, c * P:(c + 1) * P],
                                     q2aug[:, ns:ns + nsz],
                                     start=True, stop=True)
                return ps_s2

            def exp2(ps_s2, c, use_act):
                a2 = work.tile([P, t_own], bf16, tag="a_t")
                if use_act:
                    nc.scalar.activation(out=a2[:], in_=ps_s2[:], func=AF.Exp,
                                         scale=sc_a[:, c:c + 1],
                                         bias=sc_b[:, c:c + 1])
                else:
                    nc.vector.tensor_scalar(
                        out=a2[:].bitcast(i16), in0=ps_s2[:],
                        scalar1=sd_a[:, c:c + 1], scalar2=sd_b[:, c:c + 1],
                        op0=ALU.mult, op1=ALU.add)
                return a2

            def pv2(a2, c):
                for (ns, nsz) in sl:
                    nc.tensor.matmul(ps_o2[:, ns:ns + nsz], v2raw[:, c, :],
                                     a2[:, ns:ns + nsz],
                                     start=(c == 0), stop=(c == 2 * OC - 1))

            def self_attn_half(cs, n_act):
                prev = None
                for j, c in enumerate(cs):
                    ps_s2 = scores2(c)
                    if prev is not None:
                        pv2(exp2(prev[0], prev[1], prev[2]), prev[1])
                    prev = (ps_s2, c, j < n_act)
                pv2(exp2(prev[0], prev[1], prev[2]), prev[1])

            self_attn_half(range(OC), n_act_exp2)

            # -------- partner half arrives: sum - own = other -------------
            sum_sb = sing.tile([DA + 1, t_own], bf16)
            nc.sync.dma_start(sum_sb[:], cc_out[:])
            w1m_oth = sing.tile([DA + 1, t_own], bf16)
            nc.vector.tensor_tensor(out=w1m_oth[:], in0=sum_sb[:],
                                    in1=w1maug[:], op=ALU.subtract)
            prep_half(w1m_oth[:], 1)
            self_attn_half(range(OC, 2 * OC), n_act_exp2)

            # ---------------- out-projection + fused residual ------------
            o2raw = sing.tile([DA + 1, t_own], bf16)
            nc.scalar.activation(out=o2raw[:], in_=ps_o2[:], func=AF.Copy)
            r2cols = sing.tile([P, TC], f32)
            for c in range(TC):
                pr = ps_small.tile([P, 1], bf16, tag="ps_small")
                nc.tensor.transpose(pr[:], o2raw[DA:DA + 1, c * P:(c + 1) * P],
                                    sb_one1)
                nc.vector.tensor_copy(out=r2cols[:, c:c + 1], in_=pr[:])
            rc2cols = sing.tile([P, TC], f32)
            nc.vector.reciprocal(rc2cols[:], r2cols[:])

            out_r = out.rearrange("(c p) d -> p c d", p=P)
            for i in range(TC):
                po = ps_big.tile([P, d_in], f32, tag="ps_big")
                for (ns, nsz) in _slices(d_in):
                    nc.tensor.matmul(po[:, ns:ns + nsz],
                                     o2raw[:, i * P:(i + 1) * P],
                                     sb_outw[:, ns:ns + nsz],
                                     start=True, stop=True)
                ot = outp.tile([P, d_in], f32, tag="ot")
                nc.vector.scalar_tensor_tensor(
                    out=ot[:], in0=po[:], scalar=rc2cols[:, i:i + 1],
                    in1=x_tiles[i][:], op0=ALU.mult, op1=ALU.add)
                nc.sync.dma_start(out_r[:, i, :], ot[:])

    nc.compile()
    return nc


def prep_weights(f):
    """Host-side composition of the tiny weight matrices (all fp32 numpy)."""
    g, bl = f["ln_g"], f["ln_b"]
    d_in = f["w1"].shape[1]
    da = DA
    w1g = f["w1"] * g[None, :]
    c1 = f["w1"] @ bl + f["b1"]
    q1_w = SCALE * (f["wq1"] @ w1g)                     # [64, D]
    q1_b = SCALE * (f["wq1"] @ c1 + f["bq1"])           # [64]
    s1v = q1_w.sum(axis=1)                              # [64]

    # K1S [65, 66]: keys from [enc.T ; ones], cols: 64 keys + mean-corr +
    # bias-corr rows of the score contraction.
    k1s = np.zeros((da + 1, da + 2), np.float32)
    k1s[0:da, 0:da] = f["wk1"].T
    k1s[da, 0:da] = f["bk1"]
    k1s[0:da, da] = -(f["wk1"].T @ s1v)
    k1s[da, da] = -(f["bk1"] @ s1v)
    k1s[0:da, da + 1] = f["wk1"].T @ q1_b
    k1s[da, da + 1] = f["bk1"] @ q1_b

    # fold wo1*wv1 (and bv1/bo1) into the q2/k2/v2 path: h_mid = o1e @ M1.T
    # + m_b where o1e = softmax1 @ enc.
    M1 = f["wo1"] @ f["wv1"]                            # [64, 64]
    m_b = f["wo1"] @ f["bv1"] + f["bo1"]                # [64]
    q2_w = SCALE * (f["wq2"] @ M1)
    q2_b = SCALE * (f["wq2"] @ m_b + f["bq2"])
    k2_w = f["wk2"] @ M1
    k2_b = f["wk2"] @ m_b + f["bk2"]
    v2_w = f["wv2"] @ M1
    v2_b = f["wv2"] @ m_b + f["bv2"]

    k2s = np.zeros((da + 1, da + 1), np.float32)
    k2s[0:da, 0:da] = k2_w.T
    k2s[da, 0:da] = k2_b
    k2s[0:da, da] = k2_w.T @ q2_b
    k2s[da, da] = k2_b @ q2_b

    v2s = np.zeros((da + 1, da + 1), np.float32)
    v2s[0:da, 0:da] = v2_w.T
    v2s[da, 0:da] = v2_b
    v2s[da, da] = 1.0

    out_w = RES_SCALE * (f["w2"] @ f["wo2"])            # [D, 64]
    out_b = RES_SCALE * (f["w2"] @ f["bo2"] + f["b2"])  # [D]
    outw = np.zeros((da + 1, d_in), np.float32)
    outw[0:da, :] = out_w.T
    outw[da, :] = out_b

    bf = lambda a: np.ascontiguousarray(a).astype(BF16)
    return {
        "q1s": bf(q1_w.T),
        "k1s": bf(k1s),
        "q2s": bf(q2_w.T),
        "k2s": bf(k2s),
        "v2s": bf(v2s),
        "outw": bf(outw),
        "ident": bf(np.eye(P, dtype=np.float32)),
    }


def make_in_maps(inputs, t_own=T_FULL // 2):
    """Build the per-core input dicts from the full problem inputs."""
    f = {k: np.asarray(v, np.float32) for k, v in inputs.items()}
    w = prep_weights(f)
    x = f["hidden_states"]
    enc = f["encoder_hidden_states"]
    b_count = x.shape[0]
    in_maps = []
    for c in range(2 * b_count):
        b, h = c // 2, c % 2
        xo = np.ascontiguousarray(x[b, h * t_own:(h + 1) * t_own, :])
        m = dict(w)
        m["x_nat"] = xo.astype(BF16)
        m["xT"] = np.ascontiguousarray(xo.T).astype(BF16)
        encta = np.ones((DA + 1, enc.shape[1]), np.float32)
        encta[0:DA, :] = enc[b].T
        m["encTa"] = np.ascontiguousarray(encta).astype(BF16)
        ea = np.ones((enc.shape[1], DA + 1), np.float32)
        ea[:, 0:DA] = enc[b]
        m["enca"] = ea.astype(BF16)
        in_maps.append(m)
    return in_maps


LAST_RESULT = None


def kernel(**inputs):
    global LAST_RESULT
    from concourse.bass_utils import run_bass_kernel_spmd

    t_own = T_FULL // 2
    groups = [[0, 1], [2, 3], [4, 5], [6, 7]]
    key = (t_own, S_FULL, D_IN)
    if key not in _CACHE:
        _CACHE[key] = build_program(t_own, S_FULL, D_IN, groups)
    nc = _CACHE[key]

    in_maps = make_in_maps(inputs, t_own)
    res = run_bass_kernel_spmd(nc, in_maps, core_ids=list(range(N_CORES)))
    LAST_RESULT = res

    out = np.empty((B, T_FULL, D_IN), dtype=np.float32)
    for c in range(N_CORES):
        b, h = c // 2, c % 2
        out[b, h * t_own:(h + 1) * t_own, :] = res.results[c]["out"]
    return out
